# revision 11
# baseline (speedup 1.0000x reference)
"""CBOW negative-sampling loss on 8 Trainium2 NeuronCores.

Strategy: replicate the embedding tables, data-parallel over the batch dim
(2048 of 16384 rows per core).

v2 design (from the v1 trace: DVE was the critical path at ~28us busy, plus
a 1.3us mid-stream Ln table load and an 8us serial ramp):
  - u-table stored fp8e4 (values pre-scaled x64 so they sit in e4m3's normal
    range); w-table bf16. Gather traffic 5.2MB/core vs 7.3MB in v1.
  - The 8-way context sum h moves off DVE onto the idle TensorEngine: 8
    accumulating identity-matmuls per chunk group sum the gathered fp8 u-rows
    into fp32 PSUM exactly; ACT copies PSUM->SBUF bf16 with scale 1/64.
  - DVE keeps only: m = w * h (bf16 2x mode), 3 contiguous folds, one
    fp32 TensorReduce -> raw scores.
  - one fp32 TensorReduce per group (no negate split); ACT computes
    exp(-pos) and exp(+neg) per group into ex_all, one final Ln(1+x) with
    accum_out. Both act tables (Exp, Ln) are warmed by dummy activations at
    t=0 so the 1.3us table loads overlap the preamble instead of the tail.
  - 5 gather groups of (2,4,4,4,2) chunks; per group one u-gather and one
    w-gather (10 Pool DMA_INDIRECT instructions, ~1.1us fixed cost each).
    The last group's mult/fold/reduce runs on GpSimd to shorten the DVE
    tail.
  - PE warmup matmuls keep the PE array out of its low p-state before the
    first real accumulation.

loss = sum_b softplus(-score_b) + sum_{b,k} softplus(+neg_score_bk)
"""

import sys

import numpy as np

sys.path.insert(0, "/opt/trn_rl_repo")

import ml_dtypes  # noqa: E402

from concourse import bacc, bass, mybir, tile  # noqa: E402
from concourse.bass_utils import run_bass_kernel_spmd  # noqa: E402

V, D = 100000, 128
B, C, K = 16384, 8, 5
N_CORES = 8
P = 128
B_LOC = B // N_CORES            # 2048 batch rows per core
N_CHUNK = B_LOC // P            # 16 chunks of 128 rows
U_GROUPS = (1, 3, 12)           # u-gather groups (gen cost is flat -> few)
H_GROUPS = (1, 3, 4, 4, 4)      # PE/PSUM h groups (each <=4: one bank)
W_GROUPS = (1, 3, 4, 4, 2, 2)   # w-gather/compute groups (small tail)
assert sum(U_GROUPS) == sum(H_GROUPS) == sum(W_GROUPS) == N_CHUNK
J = 1 + K                       # 6 w-rows per batch row (pos + negs)
U_SCALE = 64.0                  # host pre-scale for the fp8 u-table
PAD = 128                       # zero pad rows so degenerate contiguous
                                # window reads past row V-1 stay in-tensor

_NC_CACHE = {}


def _build_bass(debug_dump=False):
    nc = bacc.Bacc(
        "TRN2",
        target_bir_lowering=False,
        debug=False,
        dynamic_dma_scratch_size=65536,
    )

    bf16 = mybir.dt.bfloat16
    fp8 = mybir.dt.float8e4
    fp32 = mybir.dt.float32
    X = mybir.AxisListType.X
    ADD = mybir.AluOpType.add
    EXPF = mybir.ActivationFunctionType.Exp
    LNF = mybir.ActivationFunctionType.Ln
    CP = mybir.ActivationFunctionType.Copy

    emb_u = nc.dram_tensor("emb_u", [V + PAD, D], fp8, kind="ExternalInput")
    emb_w = nc.dram_tensor("emb_w", [V + PAD, D], bf16, kind="ExternalInput")
    ixu_d = nc.dram_tensor("ixu", [P, N_CHUNK * C], mybir.dt.int32, kind="ExternalInput")
    ixw_d = nc.dram_tensor("ixw", [P, N_CHUNK * J], mybir.dt.int32, kind="ExternalInput")
    ident_d = nc.dram_tensor("ident2", [P, 2 * P], fp8, kind="ExternalInput")
    loss = nc.dram_tensor("loss_part", [P, 1], fp32, kind="ExternalOutput")
    if debug_dump:
        dbg_h = nc.dram_tensor("dbg_h", [P, N_CHUNK * D], bf16, kind="ExternalOutput")
        dbg_sc = nc.dram_tensor("dbg_sc", [P, N_CHUNK * J], fp32, kind="ExternalOutput")
        dbg_ug = nc.dram_tensor("dbg_ug", [P, N_CHUNK * C * D], fp8, kind="ExternalOutput")
        dbg_wg = nc.dram_tensor("dbg_wg", [P, N_CHUNK * J * D], bf16, kind="ExternalOutput")

    with tile.TileContext(nc) as tc:
        with (
            tc.tile_pool(name="idx", bufs=1) as idx_pool,
            tc.tile_pool(name="gb", bufs=1) as gb_pool,
            tc.tile_pool(name="m", bufs=3) as m_pool,
            tc.tile_pool(name="fin", bufs=1) as fin_pool,
            tc.tile_pool(name="ps", bufs=3, space="PSUM") as ps_pool,
            tc.tile_pool(name="ps1", bufs=1, space="PSUM") as ps1_pool,
        ):
            ones = fin_pool.tile([P, 1], fp32, tag="ones")
            nc.gpsimd.memset(ones[:], 1.0)

            # --- warm the Exp act table during the preamble ---
            warm_sp = fin_pool.tile([P, 1], fp32, tag="warm_sp")
            nc.scalar.activation(out=warm_sp[:], in_=ones[:], func=EXPF)

            # --- index + stacked-identity loads (SP hw-dge queue) ---
            uix = idx_pool.tile([P, N_CHUNK * C], mybir.dt.int32, tag="uix")
            nc.sync.dma_start(out=uix[:], in_=ixu_d[:])
            wix = idx_pool.tile([P, N_CHUNK * J], mybir.dt.int32, tag="wix")
            nc.sync.dma_start(out=wix[:], in_=ixw_d[:])
            ident2 = fin_pool.tile([P, 2 * P], fp8, tag="ident2")
            nc.sync.dma_start(out=ident2[:], in_=ident_d[:])
            id3 = ident2[:].rearrange("p (t i) -> p t i", t=2)

            # --- PE warmup: keep the array out of its low p-state ---
            warm_ps = ps1_pool.tile([P, P], fp32, space="PSUM")
            for _ in range(12):
                nc.tensor.matmul(
                    out=warm_ps[:], lhsT=id3, rhs=id3,
                    start=True, stop=True,
                    perf_mode=mybir.MatmulPerfMode.DoubleRow,
                )

            # --- gathered data (whole-kernel tiles; slices per group) ---
            ug_all = gb_pool.tile([P, N_CHUNK * C * D], fp8, tag="ug")
            wg_all = gb_pool.tile([P, N_CHUNK * J * D], bf16, tag="wg")
            h_all = gb_pool.tile([P, N_CHUNK * D], bf16, tag="h")
            sc_all = fin_pool.tile([P, N_CHUNK * J], fp32, tag="sc")
            # exp(-pos) in cols [0:16), exp(+neg) in cols [16:96)
            ex_all = fin_pool.tile([P, N_CHUNK * J], fp32, tag="ex_all")

            # --- gather issue (Pool): gen serialization paces everything ---
            NU, NH, NW = len(U_GROUPS), len(H_GROUPS), len(W_GROUPS)
            ustarts = [sum(U_GROUPS[:g]) for g in range(NU)]
            hstarts = [sum(H_GROUPS[:g]) for g in range(NH)]
            wstarts = [sum(W_GROUPS[:g]) for g in range(NW)]

            def gen_u(g):
                n, c0 = U_GROUPS[g], ustarts[g]
                nc.gpsimd.indirect_dma_start(
                    out=ug_all[:, c0 * C * D : (c0 + n) * C * D],
                    out_offset=None,
                    in_=emb_u[:],
                    in_offset=bass.IndirectOffsetOnAxis(
                        ap=uix[:, c0 * C : (c0 + n) * C], axis=0
                    ),
                )

            def gen_w(g):
                n, c0 = W_GROUPS[g], wstarts[g]
                nc.gpsimd.indirect_dma_start(
                    out=wg_all[:, c0 * J * D : (c0 + n) * J * D],
                    out_offset=None,
                    in_=emb_w[:],
                    in_offset=bass.IndirectOffsetOnAxis(
                        ap=wix[:, c0 * J : (c0 + n) * J], axis=0
                    ),
                )

            for which, g in [("u", 0), ("w", 0), ("u", 1), ("w", 1), ("u", 2),
                             ("w", 2), ("w", 3), ("w", 4), ("w", 5)]:
                (gen_u if which == "u" else gen_w)(g)

            # --- PE h accumulation (fp8 DoubleRow: 2 ctx rows/pass) ---
            # ug is j-major per U group: offset = cu0*C*D + j*nu*D + c*D + d,
            # so a ctx-pair slice is [p][2][n*D] with contiguous cols.
            def u_group_of(c0, n):
                for gu in range(NU):
                    if ustarts[gu] <= c0 and c0 + n <= ustarts[gu] + U_GROUPS[gu]:
                        return gu
                raise AssertionError

            for g in range(NH):
                n, c0 = H_GROUPS[g], hstarts[g]
                gu = u_group_of(c0, n)
                nu, cu0 = U_GROUPS[gu], ustarts[gu]
                base = cu0 * C * D
                hps = ps_pool.tile([P, 4 * D], fp32, space="PSUM", tag="hps")
                for t in range(C // 2):
                    pair = ug_all[
                        :, base + 2 * t * nu * D : base + (2 * t + 2) * nu * D
                    ].rearrange("p (j e) -> p j e", j=2)
                    nc.tensor.matmul(
                        out=hps[:, 0 : n * D],
                        lhsT=id3,
                        rhs=pair[:, :, (c0 - cu0) * D : (c0 - cu0 + n) * D],
                        start=(t == 0),
                        stop=(t == C // 2 - 1),
                        perf_mode=mybir.MatmulPerfMode.DoubleRow,
                    )
                nc.scalar.activation(
                    out=h_all[:, c0 * D : (c0 + n) * D],
                    in_=hps[:, 0 : n * D],
                    func=CP,
                    scale=1.0 / U_SCALE,
                )

            # --- DVE mult/folds/reduce + ACT exps per W group ---
            for g in range(NW):
                n, c0 = W_GROUPS[g], wstarts[g]
                w4 = wg_all[:, c0 * J * D : (c0 + n) * J * D].rearrange(
                    "p (c j d) -> p c j d", c=n, j=J
                )
                h4 = h_all[:, c0 * D : (c0 + n) * D].rearrange(
                    "p (c d) -> p c d", c=n
                )
                m = m_pool.tile([P, 4 * J * D], bf16, tag="m")
                m4 = m[:, 0 : n * J * D].rearrange("p (c j d) -> p c j d", c=n, j=J)
                nc.vector.tensor_mul(
                    out=m4,
                    in0=w4,
                    in1=h4[:, :, None, :].broadcast_to([P, n, J, D]),
                )
                for w_ in (64, 32, 16):
                    nc.vector.tensor_add(
                        out=m4[:, :, :, 0:w_],
                        in0=m4[:, :, :, 0:w_],
                        in1=m4[:, :, :, w_ : 2 * w_],
                    )
                sc3 = sc_all[:, c0 * J : (c0 + n) * J].rearrange(
                    "p (c j) -> p c j", j=J
                )
                nc.vector.tensor_reduce(
                    out=sc3, in_=m4[:, :, :, 0:16], axis=X, op=ADD
                )
                nc.scalar.activation(
                    out=ex_all[:, c0 : c0 + n],
                    in_=sc3[:, :, 0:1],
                    func=EXPF,
                    scale=-1.0,
                )
                nc.scalar.activation(
                    out=ex_all[:, N_CHUNK + c0 * K : N_CHUNK + (c0 + n) * K],
                    in_=sc3[:, :, 1:J],
                    func=EXPF,
                    scale=1.0,
                )

            # --- finale: ln(1+ex) accumulated per partition; host sums the
            # 128 partials per core ---
            sp = fin_pool.tile([P, N_CHUNK * J], fp32, tag="sp")
            lp1 = fin_pool.tile([P, 1], fp32, tag="lp1")
            nc.scalar.activation(
                out=sp[:], in_=ex_all[:], func=LNF, bias=1.0, accum_out=lp1[:]
            )
            nc.sync.dma_start(out=loss[:], in_=lp1[:])
            if debug_dump:
                nc.sync.dma_start(out=dbg_h[:], in_=h_all[:])
                nc.sync.dma_start(out=dbg_sc[:], in_=sc_all[:])
                nc.sync.dma_start(out=dbg_ug[:], in_=ug_all[:])
                nc.sync.dma_start(out=dbg_wg[:], in_=wg_all[:])

    nc.compile()
    return nc


def _get_nc():
    if "nc" not in _NC_CACHE:
        _NC_CACHE["nc"] = _build_bass()
    return _NC_CACHE["nc"]


def _make_in_maps(pos_u, pos_w, neg_w, u_emb, w_emb):
    pos_u = np.asarray(pos_u).astype(np.int32)
    pos_w = np.asarray(pos_w).astype(np.int32)
    neg_w = np.asarray(neg_w).astype(np.int32)
    u_emb = np.asarray(u_emb, dtype=np.float32)
    w_emb = np.asarray(w_emb, dtype=np.float32)

    emb_u = np.zeros((V + PAD, D), dtype=ml_dtypes.float8_e4m3)
    emb_u[:V] = (u_emb * U_SCALE).astype(ml_dtypes.float8_e4m3)
    emb_w = np.zeros((V + PAD, D), dtype=ml_dtypes.bfloat16)
    emb_w[:V] = w_emb.astype(ml_dtypes.bfloat16)
    ident2 = np.concatenate([np.eye(P), np.eye(P)], axis=1).astype(
        ml_dtypes.float8_e4m3
    )

    in_maps = []
    for i in range(N_CORES):
        sl = slice(i * B_LOC, (i + 1) * B_LOC)
        # batch row b -> (chunk c = b // 128, partition p = b % 128).
        # u indices are j-major within each U gather group.
        ucj = pos_u[sl].reshape(N_CHUNK, P, C).transpose(1, 2, 0)  # [P, C, N_CHUNK]
        parts = []
        cu0 = 0
        for nu in U_GROUPS:
            parts.append(ucj[:, :, cu0 : cu0 + nu].reshape(P, C * nu))
            cu0 += nu
        uix = np.concatenate(parts, axis=1)
        wrows = np.concatenate([pos_w[sl, None], neg_w[sl]], axis=1)  # [B_LOC, 6]
        wix = (
            wrows.reshape(N_CHUNK, P, J).transpose(1, 0, 2).reshape(P, N_CHUNK * J)
        )
        in_maps.append(
            {
                "emb_u": emb_u,
                "emb_w": emb_w,
                "ixu": np.ascontiguousarray(uix),
                "ixw": np.ascontiguousarray(wix),
                "ident2": ident2,
            }
        )
    return in_maps


def _install_axon_profile_shim():
    """Provide antenv.axon_hooks (missing in this image) so trace=True can
    capture NTFF profiles via the axon PJRT .so, and keep trace artifacts
    local instead of uploading to a bucket."""
    import contextlib
    import ctypes
    import types

    import concourse.bass_utils as bu

    bu.upload_artifacts = lambda tmpdir: tmpdir

    try:
        from antenv.axon_hooks import get_axon_ntff_profile_hook  # noqa: F401

        return
    except ImportError:
        pass

    mod = types.ModuleType("antenv.axon_hooks")
    holder = {}
    mod.set_axon_ntff_profile_hook = lambda h: holder.__setitem__("h", h)
    mod.get_axon_ntff_profile_hook = lambda: holder.get("h")
    sys.modules["antenv.axon_hooks"] = mod
    import antenv

    antenv.axon_hooks = mod

    so_path = "/opt/axon/libaxon_pjrt.so"
    lib = ctypes.CDLL(so_path)
    if not hasattr(lib, "axon_start_nrt_profile"):
        return
    lib.axon_start_nrt_profile.argtypes = [
        ctypes.POINTER(ctypes.c_int64),
        ctypes.c_size_t,
    ]
    lib.axon_start_nrt_profile.restype = ctypes.c_int64
    lib.axon_stop_nrt_profile.argtypes = [ctypes.c_char_p]
    lib.axon_stop_nrt_profile.restype = ctypes.c_int64

    @contextlib.contextmanager
    def _hook(output_dir, device_ids):
        import jax

        jax.devices()
        if device_ids:
            ids = (ctypes.c_int64 * len(device_ids))(*device_ids)
            rc = lib.axon_start_nrt_profile(ids, len(device_ids))
        else:
            rc = lib.axon_start_nrt_profile(None, 0)
        if rc != 0:
            raise RuntimeError(f"axon_start_nrt_profile rc={rc}")
        try:
            yield
        finally:
            n = lib.axon_stop_nrt_profile(str(output_dir).encode())
            print(f"profile: {n} file(s) written to {output_dir}")

    mod.set_axon_ntff_profile_hook(_hook)


def _run(in_maps, trace=False):
    if trace:
        _install_axon_profile_shim()
    nc = _get_nc()
    return run_bass_kernel_spmd(nc, in_maps, list(range(N_CORES)), trace=trace)


def kernel(pos_u, pos_w, neg_w, u_emb, w_emb):
    in_maps = _make_in_maps(pos_u, pos_w, neg_w, u_emb, w_emb)
    bkr = _run(in_maps, trace=False)
    total = 0.0
    for r in bkr.results:
        total += float(r["loss_part"].astype(np.float64).sum())
    return np.float32(total)


def kernel_traced(pos_u, pos_w, neg_w, u_emb, w_emb):
    """Like kernel() but returns (loss, BassKernelResults) with HW profile."""
    in_maps = _make_in_maps(pos_u, pos_w, neg_w, u_emb, w_emb)
    bkr = _run(in_maps, trace=True)
    total = 0.0
    for r in bkr.results:
        total += float(r["loss_part"].astype(np.float64).sum())
    return np.float32(total), bkr


# revision 12
# speedup vs baseline: 1.0643x; 1.0643x over previous
"""CBOW negative-sampling loss on 8 Trainium2 NeuronCores.

Strategy: replicate the embedding tables, data-parallel over the batch dim
(2048 of 16384 rows per core).

v2 design (from the v1 trace: DVE was the critical path at ~28us busy, plus
a 1.3us mid-stream Ln table load and an 8us serial ramp):
  - u-table stored fp8e4 (values pre-scaled x64 so they sit in e4m3's normal
    range); w-table bf16. Gather traffic 5.2MB/core vs 7.3MB in v1.
  - The 8-way context sum h moves off DVE onto the idle TensorEngine: 8
    accumulating identity-matmuls per chunk group sum the gathered fp8 u-rows
    into fp32 PSUM exactly; ACT copies PSUM->SBUF bf16 with scale 1/64.
  - DVE keeps only: m = w * h (bf16 2x mode), 3 contiguous folds, one
    fp32 TensorReduce -> raw scores.
  - one fp32 TensorReduce per group (no negate split); ACT computes
    exp(-pos) and exp(+neg) per group into ex_all, one final Ln(1+x) with
    accum_out. Both act tables (Exp, Ln) are warmed by dummy activations at
    t=0 so the 1.3us table loads overlap the preamble instead of the tail.
  - 5 gather groups of (2,4,4,4,2) chunks; per group one u-gather and one
    w-gather (10 Pool DMA_INDIRECT instructions, ~1.1us fixed cost each).
    The last group's mult/fold/reduce runs on GpSimd to shorten the DVE
    tail.
  - PE warmup matmuls keep the PE array out of its low p-state before the
    first real accumulation.

loss = sum_b softplus(-score_b) + sum_{b,k} softplus(+neg_score_bk)
"""

import sys

import numpy as np

sys.path.insert(0, "/opt/trn_rl_repo")

import ml_dtypes  # noqa: E402

from concourse import bacc, bass, mybir, tile  # noqa: E402
from concourse.bass_utils import run_bass_kernel_spmd  # noqa: E402

V, D = 100000, 128
B, C, K = 16384, 8, 5
N_CORES = 8
P = 128
B_LOC = B // N_CORES            # 2048 batch rows per core
N_CHUNK = B_LOC // P            # 16 chunks of 128 rows
U_GROUPS = (1, 3, 12)           # u-gather groups (gen cost is flat -> few)
H_GROUPS = (1, 3, 4, 4, 4)      # PE/PSUM h groups (each <=4: one bank)
W_GROUPS = (1, 3, 4, 4, 2, 2)   # w-gather/compute groups (small tail)
assert sum(U_GROUPS) == sum(H_GROUPS) == sum(W_GROUPS) == N_CHUNK
J = 1 + K                       # 6 w-rows per batch row (pos + negs)
U_SCALE = 64.0                  # host pre-scale for the fp8 u-table
PAD = 128                       # zero pad rows so degenerate contiguous
                                # window reads past row V-1 stay in-tensor

_NC_CACHE = {}


def _build_bass(debug_dump=False):
    nc = bacc.Bacc(
        "TRN2",
        target_bir_lowering=False,
        debug=False,
        dynamic_dma_scratch_size=65536,
    )

    bf16 = mybir.dt.bfloat16
    fp8 = mybir.dt.float8e4
    fp32 = mybir.dt.float32
    X = mybir.AxisListType.X
    ADD = mybir.AluOpType.add
    EXPF = mybir.ActivationFunctionType.Exp
    LNF = mybir.ActivationFunctionType.Ln
    CP = mybir.ActivationFunctionType.Copy

    emb_u = nc.dram_tensor("emb_u", [V + PAD, D], fp8, kind="ExternalInput")
    emb_w = nc.dram_tensor("emb_w", [V + PAD, D], bf16, kind="ExternalInput")
    ixu_d = nc.dram_tensor("ixu", [P, N_CHUNK * C], mybir.dt.int32, kind="ExternalInput")
    ixw_d = nc.dram_tensor("ixw", [P, N_CHUNK * J], mybir.dt.int32, kind="ExternalInput")
    ident_d = nc.dram_tensor("ident2", [P, 2 * P], fp8, kind="ExternalInput")
    loss = nc.dram_tensor("loss_part", [1, 1], fp32, kind="ExternalOutput")
    if debug_dump:
        dbg_h = nc.dram_tensor("dbg_h", [P, N_CHUNK * D], bf16, kind="ExternalOutput")
        dbg_sc = nc.dram_tensor("dbg_sc", [P, N_CHUNK * J], fp32, kind="ExternalOutput")
        dbg_ug = nc.dram_tensor("dbg_ug", [P, N_CHUNK * C * D], fp8, kind="ExternalOutput")
        dbg_wg = nc.dram_tensor("dbg_wg", [P, N_CHUNK * J * D], bf16, kind="ExternalOutput")

    with tile.TileContext(nc) as tc:
        with (
            tc.tile_pool(name="idx", bufs=1) as idx_pool,
            tc.tile_pool(name="gb", bufs=1) as gb_pool,
            tc.tile_pool(name="m", bufs=3) as m_pool,
            tc.tile_pool(name="fin", bufs=1) as fin_pool,
            tc.tile_pool(name="ps", bufs=3, space="PSUM") as ps_pool,
            tc.tile_pool(name="ps1", bufs=1, space="PSUM") as ps1_pool,
        ):
            ones = fin_pool.tile([P, 1], fp32, tag="ones")
            nc.gpsimd.memset(ones[:], 1.0)

            # --- warm the Exp act table during the preamble ---
            warm_sp = fin_pool.tile([P, 1], fp32, tag="warm_sp")
            nc.scalar.activation(out=warm_sp[:], in_=ones[:], func=EXPF)

            # --- index + stacked-identity loads (SP hw-dge queue) ---
            uix = idx_pool.tile([P, N_CHUNK * C], mybir.dt.int32, tag="uix")
            nc.sync.dma_start(out=uix[:], in_=ixu_d[:])
            wix = idx_pool.tile([P, N_CHUNK * J], mybir.dt.int32, tag="wix")
            nc.sync.dma_start(out=wix[:], in_=ixw_d[:])
            ident2 = fin_pool.tile([P, 2 * P], fp8, tag="ident2")
            nc.sync.dma_start(out=ident2[:], in_=ident_d[:])
            id3 = ident2[:].rearrange("p (t i) -> p t i", t=2)

            # --- PE warmup: keep the array out of its low p-state ---
            warm_ps = ps1_pool.tile([P, P], fp32, space="PSUM")
            for _ in range(12):
                nc.tensor.matmul(
                    out=warm_ps[:], lhsT=id3, rhs=id3,
                    start=True, stop=True,
                    perf_mode=mybir.MatmulPerfMode.DoubleRow,
                )

            # --- gathered data (whole-kernel tiles; slices per group) ---
            ug_all = gb_pool.tile([P, N_CHUNK * C * D], fp8, tag="ug")
            wg_all = gb_pool.tile([P, N_CHUNK * J * D], bf16, tag="wg")
            h_all = gb_pool.tile([P, N_CHUNK * D], bf16, tag="h")
            sc_all = fin_pool.tile([P, N_CHUNK * J], fp32, tag="sc")
            # exp(-pos) in cols [0:16), exp(+neg) in cols [16:96)
            ex_all = fin_pool.tile([P, N_CHUNK * J], fp32, tag="ex_all")

            # --- gather issue (Pool): gen serialization paces everything ---
            NU, NH, NW = len(U_GROUPS), len(H_GROUPS), len(W_GROUPS)
            ustarts = [sum(U_GROUPS[:g]) for g in range(NU)]
            hstarts = [sum(H_GROUPS[:g]) for g in range(NH)]
            wstarts = [sum(W_GROUPS[:g]) for g in range(NW)]

            def gen_u(g):
                n, c0 = U_GROUPS[g], ustarts[g]
                nc.gpsimd.indirect_dma_start(
                    out=ug_all[:, c0 * C * D : (c0 + n) * C * D],
                    out_offset=None,
                    in_=emb_u[:],
                    in_offset=bass.IndirectOffsetOnAxis(
                        ap=uix[:, c0 * C : (c0 + n) * C], axis=0
                    ),
                )

            def gen_w(g):
                n, c0 = W_GROUPS[g], wstarts[g]
                nc.gpsimd.indirect_dma_start(
                    out=wg_all[:, c0 * J * D : (c0 + n) * J * D],
                    out_offset=None,
                    in_=emb_w[:],
                    in_offset=bass.IndirectOffsetOnAxis(
                        ap=wix[:, c0 * J : (c0 + n) * J], axis=0
                    ),
                )

            for which, g in [("u", 0), ("w", 0), ("u", 1), ("w", 1), ("u", 2),
                             ("w", 2), ("w", 3), ("w", 4), ("w", 5)]:
                (gen_u if which == "u" else gen_w)(g)

            # --- PE h accumulation (fp8 DoubleRow: 2 ctx rows/pass) ---
            # ug is j-major per U group: offset = cu0*C*D + j*nu*D + c*D + d,
            # so a ctx-pair slice is [p][2][n*D] with contiguous cols.
            def u_group_of(c0, n):
                for gu in range(NU):
                    if ustarts[gu] <= c0 and c0 + n <= ustarts[gu] + U_GROUPS[gu]:
                        return gu
                raise AssertionError

            for g in range(NH):
                n, c0 = H_GROUPS[g], hstarts[g]
                gu = u_group_of(c0, n)
                nu, cu0 = U_GROUPS[gu], ustarts[gu]
                base = cu0 * C * D
                hps = ps_pool.tile([P, 4 * D], fp32, space="PSUM", tag="hps")
                for t in range(C // 2):
                    pair = ug_all[
                        :, base + 2 * t * nu * D : base + (2 * t + 2) * nu * D
                    ].rearrange("p (j e) -> p j e", j=2)
                    nc.tensor.matmul(
                        out=hps[:, 0 : n * D],
                        lhsT=id3,
                        rhs=pair[:, :, (c0 - cu0) * D : (c0 - cu0 + n) * D],
                        start=(t == 0),
                        stop=(t == C // 2 - 1),
                        perf_mode=mybir.MatmulPerfMode.DoubleRow,
                    )
                nc.scalar.activation(
                    out=h_all[:, c0 * D : (c0 + n) * D],
                    in_=hps[:, 0 : n * D],
                    func=CP,
                    scale=1.0 / U_SCALE,
                )

            # --- DVE mult/folds/reduce + ACT exps per W group ---
            for g in range(NW):
                n, c0 = W_GROUPS[g], wstarts[g]
                w4 = wg_all[:, c0 * J * D : (c0 + n) * J * D].rearrange(
                    "p (c j d) -> p c j d", c=n, j=J
                )
                h4 = h_all[:, c0 * D : (c0 + n) * D].rearrange(
                    "p (c d) -> p c d", c=n
                )
                m = m_pool.tile([P, 4 * J * D], bf16, tag="m")
                m4 = m[:, 0 : n * J * D].rearrange("p (c j d) -> p c j d", c=n, j=J)
                nc.vector.tensor_mul(
                    out=m4,
                    in0=w4,
                    in1=h4[:, :, None, :].broadcast_to([P, n, J, D]),
                )
                for w_ in (64, 32, 16):
                    nc.vector.tensor_add(
                        out=m4[:, :, :, 0:w_],
                        in0=m4[:, :, :, 0:w_],
                        in1=m4[:, :, :, w_ : 2 * w_],
                    )
                sc3 = sc_all[:, c0 * J : (c0 + n) * J].rearrange(
                    "p (c j) -> p c j", j=J
                )
                nc.vector.tensor_reduce(
                    out=sc3, in_=m4[:, :, :, 0:16], axis=X, op=ADD
                )
                nc.scalar.activation(
                    out=ex_all[:, c0 : c0 + n],
                    in_=sc3[:, :, 0:1],
                    func=EXPF,
                    scale=-1.0,
                )
                nc.scalar.activation(
                    out=ex_all[:, N_CHUNK + c0 * K : N_CHUNK + (c0 + n) * K],
                    in_=sc3[:, :, 1:J],
                    func=EXPF,
                    scale=1.0,
                )

            # --- finale: ln(1+ex) summed per partition, then collapse.
            # (A [P,1] output DMA would need 128 4-byte descriptors whose
            # completion lags ~7us; the ones-matmul keeps it one descriptor.)
            sp = fin_pool.tile([P, N_CHUNK * J], fp32, tag="sp")
            lp1 = fin_pool.tile([P, 1], fp32, tag="lp1")
            nc.scalar.activation(
                out=sp[:], in_=ex_all[:], func=LNF, bias=1.0, accum_out=lp1[:]
            )
            acc = ps1_pool.tile([1, 1], fp32, space="PSUM")
            nc.tensor.matmul(out=acc[:], lhsT=ones[:], rhs=lp1[:], start=True, stop=True)
            out_sb = fin_pool.tile([1, 1], fp32, tag="out")
            nc.vector.tensor_copy(out=out_sb[:], in_=acc[:])
            nc.sync.dma_start(out=loss[:], in_=out_sb[:])
            if debug_dump:
                nc.sync.dma_start(out=dbg_h[:], in_=h_all[:])
                nc.sync.dma_start(out=dbg_sc[:], in_=sc_all[:])
                nc.sync.dma_start(out=dbg_ug[:], in_=ug_all[:])
                nc.sync.dma_start(out=dbg_wg[:], in_=wg_all[:])

    nc.compile()
    return nc


def _get_nc():
    if "nc" not in _NC_CACHE:
        _NC_CACHE["nc"] = _build_bass()
    return _NC_CACHE["nc"]


def _make_in_maps(pos_u, pos_w, neg_w, u_emb, w_emb):
    pos_u = np.asarray(pos_u).astype(np.int32)
    pos_w = np.asarray(pos_w).astype(np.int32)
    neg_w = np.asarray(neg_w).astype(np.int32)
    u_emb = np.asarray(u_emb, dtype=np.float32)
    w_emb = np.asarray(w_emb, dtype=np.float32)

    emb_u = np.zeros((V + PAD, D), dtype=ml_dtypes.float8_e4m3)
    emb_u[:V] = (u_emb * U_SCALE).astype(ml_dtypes.float8_e4m3)
    emb_w = np.zeros((V + PAD, D), dtype=ml_dtypes.bfloat16)
    emb_w[:V] = w_emb.astype(ml_dtypes.bfloat16)
    ident2 = np.concatenate([np.eye(P), np.eye(P)], axis=1).astype(
        ml_dtypes.float8_e4m3
    )

    in_maps = []
    for i in range(N_CORES):
        sl = slice(i * B_LOC, (i + 1) * B_LOC)
        # batch row b -> (chunk c = b // 128, partition p = b % 128).
        # u indices are j-major within each U gather group.
        ucj = pos_u[sl].reshape(N_CHUNK, P, C).transpose(1, 2, 0)  # [P, C, N_CHUNK]
        parts = []
        cu0 = 0
        for nu in U_GROUPS:
            parts.append(ucj[:, :, cu0 : cu0 + nu].reshape(P, C * nu))
            cu0 += nu
        uix = np.concatenate(parts, axis=1)
        wrows = np.concatenate([pos_w[sl, None], neg_w[sl]], axis=1)  # [B_LOC, 6]
        wix = (
            wrows.reshape(N_CHUNK, P, J).transpose(1, 0, 2).reshape(P, N_CHUNK * J)
        )
        in_maps.append(
            {
                "emb_u": emb_u,
                "emb_w": emb_w,
                "ixu": np.ascontiguousarray(uix),
                "ixw": np.ascontiguousarray(wix),
                "ident2": ident2,
            }
        )
    return in_maps


def _install_axon_profile_shim():
    """Provide antenv.axon_hooks (missing in this image) so trace=True can
    capture NTFF profiles via the axon PJRT .so, and keep trace artifacts
    local instead of uploading to a bucket."""
    import contextlib
    import ctypes
    import types

    import concourse.bass_utils as bu

    bu.upload_artifacts = lambda tmpdir: tmpdir

    try:
        from antenv.axon_hooks import get_axon_ntff_profile_hook  # noqa: F401

        return
    except ImportError:
        pass

    mod = types.ModuleType("antenv.axon_hooks")
    holder = {}
    mod.set_axon_ntff_profile_hook = lambda h: holder.__setitem__("h", h)
    mod.get_axon_ntff_profile_hook = lambda: holder.get("h")
    sys.modules["antenv.axon_hooks"] = mod
    import antenv

    antenv.axon_hooks = mod

    so_path = "/opt/axon/libaxon_pjrt.so"
    lib = ctypes.CDLL(so_path)
    if not hasattr(lib, "axon_start_nrt_profile"):
        return
    lib.axon_start_nrt_profile.argtypes = [
        ctypes.POINTER(ctypes.c_int64),
        ctypes.c_size_t,
    ]
    lib.axon_start_nrt_profile.restype = ctypes.c_int64
    lib.axon_stop_nrt_profile.argtypes = [ctypes.c_char_p]
    lib.axon_stop_nrt_profile.restype = ctypes.c_int64

    @contextlib.contextmanager
    def _hook(output_dir, device_ids):
        import jax

        jax.devices()
        if device_ids:
            ids = (ctypes.c_int64 * len(device_ids))(*device_ids)
            rc = lib.axon_start_nrt_profile(ids, len(device_ids))
        else:
            rc = lib.axon_start_nrt_profile(None, 0)
        if rc != 0:
            raise RuntimeError(f"axon_start_nrt_profile rc={rc}")
        try:
            yield
        finally:
            n = lib.axon_stop_nrt_profile(str(output_dir).encode())
            print(f"profile: {n} file(s) written to {output_dir}")

    mod.set_axon_ntff_profile_hook(_hook)


def _run(in_maps, trace=False):
    if trace:
        _install_axon_profile_shim()
    nc = _get_nc()
    return run_bass_kernel_spmd(nc, in_maps, list(range(N_CORES)), trace=trace)


def kernel(pos_u, pos_w, neg_w, u_emb, w_emb):
    in_maps = _make_in_maps(pos_u, pos_w, neg_w, u_emb, w_emb)
    bkr = _run(in_maps, trace=False)
    total = 0.0
    for r in bkr.results:
        total += float(r["loss_part"].astype(np.float64).sum())
    return np.float32(total)


def kernel_traced(pos_u, pos_w, neg_w, u_emb, w_emb):
    """Like kernel() but returns (loss, BassKernelResults) with HW profile."""
    in_maps = _make_in_maps(pos_u, pos_w, neg_w, u_emb, w_emb)
    bkr = _run(in_maps, trace=True)
    total = 0.0
    for r in bkr.results:
        total += float(r["loss_part"].astype(np.float64).sum())
    return np.float32(total), bkr


# revision 13
# speedup vs baseline: 1.2213x; 1.1475x over previous
"""CBOW negative-sampling loss on 8 Trainium2 NeuronCores.

Strategy: replicate the embedding tables, data-parallel over the batch dim
(2048 of 16384 rows per core).

v2 design (from the v1 trace: DVE was the critical path at ~28us busy, plus
a 1.3us mid-stream Ln table load and an 8us serial ramp):
  - u-table stored fp8e4 (values pre-scaled x64 so they sit in e4m3's normal
    range); w-table bf16. Gather traffic 5.2MB/core vs 7.3MB in v1.
  - The 8-way context sum h moves off DVE onto the idle TensorEngine: 8
    accumulating identity-matmuls per chunk group sum the gathered fp8 u-rows
    into fp32 PSUM exactly; ACT copies PSUM->SBUF bf16 with scale 1/64.
  - DVE keeps only: m = w * h (bf16 2x mode), 3 contiguous folds, one
    fp32 TensorReduce -> raw scores.
  - one fp32 TensorReduce per group (no negate split); ACT computes
    exp(-pos) and exp(+neg) per group into ex_all, one final Ln(1+x) with
    accum_out. Both act tables (Exp, Ln) are warmed by dummy activations at
    t=0 so the 1.3us table loads overlap the preamble instead of the tail.
  - 5 gather groups of (2,4,4,4,2) chunks; per group one u-gather and one
    w-gather (10 Pool DMA_INDIRECT instructions, ~1.1us fixed cost each).
    The last group's mult/fold/reduce runs on GpSimd to shorten the DVE
    tail.
  - PE warmup matmuls keep the PE array out of its low p-state before the
    first real accumulation.

loss = sum_b softplus(-score_b) + sum_{b,k} softplus(+neg_score_bk)
"""

import sys

import numpy as np

sys.path.insert(0, "/opt/trn_rl_repo")

import ml_dtypes  # noqa: E402

from concourse import bacc, bass, mybir, tile  # noqa: E402
from concourse.bass_utils import run_bass_kernel_spmd  # noqa: E402

V, D = 100000, 128
B, C, K = 16384, 8, 5
N_CORES = 8
P = 128
B_LOC = B // N_CORES            # 2048 batch rows per core
N_CHUNK = B_LOC // P            # 16 chunks of 128 rows
U_GROUPS = (1, 3, 4, 8)         # u-gather groups
H_GROUPS = (1, 3, 4, 4, 4)      # PE/PSUM h groups (each <=4: one bank)
W_GROUPS = (1, 3, 4, 4, 2, 2)   # w-gather/compute groups (small tail)
assert sum(U_GROUPS) == sum(H_GROUPS) == sum(W_GROUPS) == N_CHUNK
J = 1 + K                       # 6 w-rows per batch row (pos + negs)
U_SCALE = 64.0                  # host pre-scale for the fp8 u-table
PAD = 128                       # zero pad rows so degenerate contiguous
                                # window reads past row V-1 stay in-tensor

_NC_CACHE = {}


def _build_bass(debug_dump=False):
    nc = bacc.Bacc(
        "TRN2",
        target_bir_lowering=False,
        debug=False,
        dynamic_dma_scratch_size=65536,
    )

    bf16 = mybir.dt.bfloat16
    fp8 = mybir.dt.float8e4
    fp32 = mybir.dt.float32
    X = mybir.AxisListType.X
    ADD = mybir.AluOpType.add
    EXPF = mybir.ActivationFunctionType.Exp
    LNF = mybir.ActivationFunctionType.Ln
    CP = mybir.ActivationFunctionType.Copy

    emb_u = nc.dram_tensor("emb_u", [V + PAD, D], fp8, kind="ExternalInput")
    emb_w = nc.dram_tensor("emb_w", [V + PAD, D], bf16, kind="ExternalInput")
    ixu_d = nc.dram_tensor("ixu", [P, N_CHUNK * C], mybir.dt.int32, kind="ExternalInput")
    ixw_d = nc.dram_tensor("ixw", [P, N_CHUNK * J], mybir.dt.int32, kind="ExternalInput")
    ident_d = nc.dram_tensor("ident2", [P, 2 * P], fp8, kind="ExternalInput")
    loss = nc.dram_tensor("loss_part", [1, 1], fp32, kind="ExternalOutput")
    if debug_dump:
        dbg_h = nc.dram_tensor("dbg_h", [P, N_CHUNK * D], bf16, kind="ExternalOutput")
        dbg_sc = nc.dram_tensor("dbg_sc", [P, N_CHUNK * J], fp32, kind="ExternalOutput")
        dbg_ug = nc.dram_tensor("dbg_ug", [P, N_CHUNK * C * D], fp8, kind="ExternalOutput")
        dbg_wg = nc.dram_tensor("dbg_wg", [P, N_CHUNK * J * D], bf16, kind="ExternalOutput")

    with tile.TileContext(nc) as tc:
        with (
            tc.tile_pool(name="idx", bufs=1) as idx_pool,
            tc.tile_pool(name="gb", bufs=1) as gb_pool,
            tc.tile_pool(name="m", bufs=3) as m_pool,
            tc.tile_pool(name="fin", bufs=1) as fin_pool,
            tc.tile_pool(name="ps", bufs=3, space="PSUM") as ps_pool,
            tc.tile_pool(name="ps1", bufs=1, space="PSUM") as ps1_pool,
        ):
            ones = fin_pool.tile([P, 1], fp32, tag="ones")
            nc.gpsimd.memset(ones[:], 1.0)

            # --- warm the Exp act table during the preamble ---
            warm_sp = fin_pool.tile([P, 1], fp32, tag="warm_sp")
            nc.scalar.activation(out=warm_sp[:], in_=ones[:], func=EXPF)

            # --- index + stacked-identity loads (SP hw-dge queue) ---
            uix = idx_pool.tile([P, N_CHUNK * C], mybir.dt.int32, tag="uix")
            nc.sync.dma_start(out=uix[:], in_=ixu_d[:])
            wix = idx_pool.tile([P, N_CHUNK * J], mybir.dt.int32, tag="wix")
            nc.sync.dma_start(out=wix[:], in_=ixw_d[:])
            ident2 = fin_pool.tile([P, 2 * P], fp8, tag="ident2")
            nc.sync.dma_start(out=ident2[:], in_=ident_d[:])
            id3 = ident2[:].rearrange("p (t i) -> p t i", t=2)

            # --- PE warmup: keep the array out of its low p-state ---
            warm_ps = ps1_pool.tile([P, P], fp32, space="PSUM")
            for _ in range(12):
                nc.tensor.matmul(
                    out=warm_ps[:], lhsT=id3, rhs=id3,
                    start=True, stop=True,
                    perf_mode=mybir.MatmulPerfMode.DoubleRow,
                )

            # --- gathered data (whole-kernel tiles; slices per group) ---
            ug_all = gb_pool.tile([P, N_CHUNK * C * D], fp8, tag="ug")
            wg_all = gb_pool.tile([P, N_CHUNK * J * D], bf16, tag="wg")
            h_all = gb_pool.tile([P, N_CHUNK * D], bf16, tag="h")
            sc_all = fin_pool.tile([P, N_CHUNK * J], fp32, tag="sc")
            # exp(-pos) in cols [0:16), exp(+neg) in cols [16:96)
            ex_all = fin_pool.tile([P, N_CHUNK * J], fp32, tag="ex_all")

            # --- gather issue (Pool): gen serialization paces everything ---
            NU, NH, NW = len(U_GROUPS), len(H_GROUPS), len(W_GROUPS)
            ustarts = [sum(U_GROUPS[:g]) for g in range(NU)]
            hstarts = [sum(H_GROUPS[:g]) for g in range(NH)]
            wstarts = [sum(W_GROUPS[:g]) for g in range(NW)]

            def gen_u(g):
                n, c0 = U_GROUPS[g], ustarts[g]
                nc.gpsimd.indirect_dma_start(
                    out=ug_all[:, c0 * C * D : (c0 + n) * C * D],
                    out_offset=None,
                    in_=emb_u[:],
                    in_offset=bass.IndirectOffsetOnAxis(
                        ap=uix[:, c0 * C : (c0 + n) * C], axis=0
                    ),
                )

            def gen_w(g):
                n, c0 = W_GROUPS[g], wstarts[g]
                nc.gpsimd.indirect_dma_start(
                    out=wg_all[:, c0 * J * D : (c0 + n) * J * D],
                    out_offset=None,
                    in_=emb_w[:],
                    in_offset=bass.IndirectOffsetOnAxis(
                        ap=wix[:, c0 * J : (c0 + n) * J], axis=0
                    ),
                )

            for which, g in [("u", 0), ("w", 0), ("u", 1), ("w", 1), ("u", 2),
                             ("w", 2), ("u", 3), ("w", 3), ("w", 4), ("w", 5)]:
                (gen_u if which == "u" else gen_w)(g)

            # --- PE h accumulation (fp8 DoubleRow: 2 ctx rows/pass) ---
            # ug is j-major per U group: offset = cu0*C*D + j*nu*D + c*D + d,
            # so a ctx-pair slice is [p][2][n*D] with contiguous cols.
            def u_group_of(c0, n):
                for gu in range(NU):
                    if ustarts[gu] <= c0 and c0 + n <= ustarts[gu] + U_GROUPS[gu]:
                        return gu
                raise AssertionError

            for g in range(NH):
                n, c0 = H_GROUPS[g], hstarts[g]
                gu = u_group_of(c0, n)
                nu, cu0 = U_GROUPS[gu], ustarts[gu]
                base = cu0 * C * D
                hps = ps_pool.tile([P, 4 * D], fp32, space="PSUM", tag="hps")
                for t in range(C // 2):
                    pair = ug_all[
                        :, base + 2 * t * nu * D : base + (2 * t + 2) * nu * D
                    ].rearrange("p (j e) -> p j e", j=2)
                    nc.tensor.matmul(
                        out=hps[:, 0 : n * D],
                        lhsT=id3,
                        rhs=pair[:, :, (c0 - cu0) * D : (c0 - cu0 + n) * D],
                        start=(t == 0),
                        stop=(t == C // 2 - 1),
                        perf_mode=mybir.MatmulPerfMode.DoubleRow,
                    )
                nc.scalar.activation(
                    out=h_all[:, c0 * D : (c0 + n) * D],
                    in_=hps[:, 0 : n * D],
                    func=CP,
                    scale=1.0 / U_SCALE,
                )

            # --- DVE mult/folds/reduce + ACT exps per W group ---
            for g in range(NW):
                n, c0 = W_GROUPS[g], wstarts[g]
                w4 = wg_all[:, c0 * J * D : (c0 + n) * J * D].rearrange(
                    "p (c j d) -> p c j d", c=n, j=J
                )
                h4 = h_all[:, c0 * D : (c0 + n) * D].rearrange(
                    "p (c d) -> p c d", c=n
                )
                m = m_pool.tile([P, 4 * J * D], bf16, tag="m")
                m4 = m[:, 0 : n * J * D].rearrange("p (c j d) -> p c j d", c=n, j=J)
                nc.vector.tensor_mul(
                    out=m4,
                    in0=w4,
                    in1=h4[:, :, None, :].broadcast_to([P, n, J, D]),
                )
                for w_ in (64, 32, 16):
                    nc.vector.tensor_add(
                        out=m4[:, :, :, 0:w_],
                        in0=m4[:, :, :, 0:w_],
                        in1=m4[:, :, :, w_ : 2 * w_],
                    )
                sc3 = sc_all[:, c0 * J : (c0 + n) * J].rearrange(
                    "p (c j) -> p c j", j=J
                )
                nc.vector.tensor_reduce(
                    out=sc3, in_=m4[:, :, :, 0:16], axis=X, op=ADD
                )
                nc.scalar.activation(
                    out=ex_all[:, c0 : c0 + n],
                    in_=sc3[:, :, 0:1],
                    func=EXPF,
                    scale=-1.0,
                )
                nc.scalar.activation(
                    out=ex_all[:, N_CHUNK + c0 * K : N_CHUNK + (c0 + n) * K],
                    in_=sc3[:, :, 1:J],
                    func=EXPF,
                    scale=1.0,
                )

            # --- finale: ln(1+ex) summed per partition, then collapse.
            # (A [P,1] output DMA would need 128 4-byte descriptors whose
            # completion lags ~7us; the ones-matmul keeps it one descriptor.)
            sp = fin_pool.tile([P, N_CHUNK * J], fp32, tag="sp")
            lp1 = fin_pool.tile([P, 1], fp32, tag="lp1")
            nc.scalar.activation(
                out=sp[:], in_=ex_all[:], func=LNF, bias=1.0, accum_out=lp1[:]
            )
            acc = ps1_pool.tile([1, 1], fp32, space="PSUM")
            nc.tensor.matmul(out=acc[:], lhsT=ones[:], rhs=lp1[:], start=True, stop=True)
            out_sb = fin_pool.tile([1, 1], fp32, tag="out")
            nc.vector.tensor_copy(out=out_sb[:], in_=acc[:])
            nc.sync.dma_start(out=loss[:], in_=out_sb[:])
            if debug_dump:
                nc.sync.dma_start(out=dbg_h[:], in_=h_all[:])
                nc.sync.dma_start(out=dbg_sc[:], in_=sc_all[:])
                nc.sync.dma_start(out=dbg_ug[:], in_=ug_all[:])
                nc.sync.dma_start(out=dbg_wg[:], in_=wg_all[:])

    nc.compile()
    return nc


def _get_nc():
    if "nc" not in _NC_CACHE:
        _NC_CACHE["nc"] = _build_bass()
    return _NC_CACHE["nc"]


def _make_in_maps(pos_u, pos_w, neg_w, u_emb, w_emb):
    pos_u = np.asarray(pos_u).astype(np.int32)
    pos_w = np.asarray(pos_w).astype(np.int32)
    neg_w = np.asarray(neg_w).astype(np.int32)
    u_emb = np.asarray(u_emb, dtype=np.float32)
    w_emb = np.asarray(w_emb, dtype=np.float32)

    emb_u = np.zeros((V + PAD, D), dtype=ml_dtypes.float8_e4m3)
    emb_u[:V] = (u_emb * U_SCALE).astype(ml_dtypes.float8_e4m3)
    emb_w = np.zeros((V + PAD, D), dtype=ml_dtypes.bfloat16)
    emb_w[:V] = w_emb.astype(ml_dtypes.bfloat16)
    ident2 = np.concatenate([np.eye(P), np.eye(P)], axis=1).astype(
        ml_dtypes.float8_e4m3
    )

    in_maps = []
    for i in range(N_CORES):
        sl = slice(i * B_LOC, (i + 1) * B_LOC)
        # batch row b -> (chunk c = b // 128, partition p = b % 128).
        # u indices are j-major within each U gather group.
        ucj = pos_u[sl].reshape(N_CHUNK, P, C).transpose(1, 2, 0)  # [P, C, N_CHUNK]
        parts = []
        cu0 = 0
        for nu in U_GROUPS:
            parts.append(ucj[:, :, cu0 : cu0 + nu].reshape(P, C * nu))
            cu0 += nu
        uix = np.concatenate(parts, axis=1)
        wrows = np.concatenate([pos_w[sl, None], neg_w[sl]], axis=1)  # [B_LOC, 6]
        wix = (
            wrows.reshape(N_CHUNK, P, J).transpose(1, 0, 2).reshape(P, N_CHUNK * J)
        )
        in_maps.append(
            {
                "emb_u": emb_u,
                "emb_w": emb_w,
                "ixu": np.ascontiguousarray(uix),
                "ixw": np.ascontiguousarray(wix),
                "ident2": ident2,
            }
        )
    return in_maps


def _install_axon_profile_shim():
    """Provide antenv.axon_hooks (missing in this image) so trace=True can
    capture NTFF profiles via the axon PJRT .so, and keep trace artifacts
    local instead of uploading to a bucket."""
    import contextlib
    import ctypes
    import types

    import concourse.bass_utils as bu

    bu.upload_artifacts = lambda tmpdir: tmpdir

    try:
        from antenv.axon_hooks import get_axon_ntff_profile_hook  # noqa: F401

        return
    except ImportError:
        pass

    mod = types.ModuleType("antenv.axon_hooks")
    holder = {}
    mod.set_axon_ntff_profile_hook = lambda h: holder.__setitem__("h", h)
    mod.get_axon_ntff_profile_hook = lambda: holder.get("h")
    sys.modules["antenv.axon_hooks"] = mod
    import antenv

    antenv.axon_hooks = mod

    so_path = "/opt/axon/libaxon_pjrt.so"
    lib = ctypes.CDLL(so_path)
    if not hasattr(lib, "axon_start_nrt_profile"):
        return
    lib.axon_start_nrt_profile.argtypes = [
        ctypes.POINTER(ctypes.c_int64),
        ctypes.c_size_t,
    ]
    lib.axon_start_nrt_profile.restype = ctypes.c_int64
    lib.axon_stop_nrt_profile.argtypes = [ctypes.c_char_p]
    lib.axon_stop_nrt_profile.restype = ctypes.c_int64

    @contextlib.contextmanager
    def _hook(output_dir, device_ids):
        import jax

        jax.devices()
        if device_ids:
            ids = (ctypes.c_int64 * len(device_ids))(*device_ids)
            rc = lib.axon_start_nrt_profile(ids, len(device_ids))
        else:
            rc = lib.axon_start_nrt_profile(None, 0)
        if rc != 0:
            raise RuntimeError(f"axon_start_nrt_profile rc={rc}")
        try:
            yield
        finally:
            n = lib.axon_stop_nrt_profile(str(output_dir).encode())
            print(f"profile: {n} file(s) written to {output_dir}")

    mod.set_axon_ntff_profile_hook(_hook)


def _run(in_maps, trace=False):
    if trace:
        _install_axon_profile_shim()
    nc = _get_nc()
    return run_bass_kernel_spmd(nc, in_maps, list(range(N_CORES)), trace=trace)


def kernel(pos_u, pos_w, neg_w, u_emb, w_emb):
    in_maps = _make_in_maps(pos_u, pos_w, neg_w, u_emb, w_emb)
    bkr = _run(in_maps, trace=False)
    total = 0.0
    for r in bkr.results:
        total += float(r["loss_part"].astype(np.float64).sum())
    return np.float32(total)


def kernel_traced(pos_u, pos_w, neg_w, u_emb, w_emb):
    """Like kernel() but returns (loss, BassKernelResults) with HW profile."""
    in_maps = _make_in_maps(pos_u, pos_w, neg_w, u_emb, w_emb)
    bkr = _run(in_maps, trace=True)
    total = 0.0
    for r in bkr.results:
        total += float(r["loss_part"].astype(np.float64).sum())
    return np.float32(total), bkr


# revision 14
# speedup vs baseline: 1.2415x; 1.0165x over previous
"""CBOW negative-sampling loss on 8 Trainium2 NeuronCores.

Strategy: replicate the embedding tables, data-parallel over the batch dim
(2048 of 16384 rows per core).

v2 design (from the v1 trace: DVE was the critical path at ~28us busy, plus
a 1.3us mid-stream Ln table load and an 8us serial ramp):
  - u-table stored fp8e4 (values pre-scaled x64 so they sit in e4m3's normal
    range); w-table bf16. Gather traffic 5.2MB/core vs 7.3MB in v1.
  - The 8-way context sum h moves off DVE onto the idle TensorEngine: 8
    accumulating identity-matmuls per chunk group sum the gathered fp8 u-rows
    into fp32 PSUM exactly; ACT copies PSUM->SBUF bf16 with scale 1/64.
  - DVE keeps only: m = w * h (bf16 2x mode), 3 contiguous folds, one
    fp32 TensorReduce -> raw scores.
  - one fp32 TensorReduce per group (no negate split); ACT computes
    exp(-pos) and exp(+neg) per group into ex_all, one final Ln(1+x) with
    accum_out. Both act tables (Exp, Ln) are warmed by dummy activations at
    t=0 so the 1.3us table loads overlap the preamble instead of the tail.
  - 5 gather groups of (2,4,4,4,2) chunks; per group one u-gather and one
    w-gather (10 Pool DMA_INDIRECT instructions, ~1.1us fixed cost each).
    The last group's mult/fold/reduce runs on GpSimd to shorten the DVE
    tail.
  - PE warmup matmuls keep the PE array out of its low p-state before the
    first real accumulation.

loss = sum_b softplus(-score_b) + sum_{b,k} softplus(+neg_score_bk)
"""

import sys

import numpy as np

sys.path.insert(0, "/opt/trn_rl_repo")

import ml_dtypes  # noqa: E402

from concourse import bacc, bass, mybir, tile  # noqa: E402
from concourse.bass_utils import run_bass_kernel_spmd  # noqa: E402

V, D = 100000, 128
B, C, K = 16384, 8, 5
N_CORES = 8
P = 128
B_LOC = B // N_CORES            # 2048 batch rows per core
N_CHUNK = B_LOC // P            # 16 chunks of 128 rows
U_GROUPS = (1, 3, 4, 4, 4)      # u-gather groups
H_GROUPS = (1, 3, 4, 4, 4)      # PE/PSUM h groups (each <=4: one bank)
W_GROUPS = (1, 3, 4, 4, 2, 2)   # w-gather/compute groups (small tail)
assert sum(U_GROUPS) == sum(H_GROUPS) == sum(W_GROUPS) == N_CHUNK
J = 1 + K                       # 6 w-rows per batch row (pos + negs)
U_SCALE = 64.0                  # host pre-scale for the fp8 u-table
PAD = 128                       # zero pad rows so degenerate contiguous
                                # window reads past row V-1 stay in-tensor

_NC_CACHE = {}


def _build_bass(debug_dump=False):
    nc = bacc.Bacc(
        "TRN2",
        target_bir_lowering=False,
        debug=False,
        dynamic_dma_scratch_size=65536,
    )

    bf16 = mybir.dt.bfloat16
    fp8 = mybir.dt.float8e4
    fp32 = mybir.dt.float32
    X = mybir.AxisListType.X
    ADD = mybir.AluOpType.add
    EXPF = mybir.ActivationFunctionType.Exp
    LNF = mybir.ActivationFunctionType.Ln
    CP = mybir.ActivationFunctionType.Copy

    emb_u = nc.dram_tensor("emb_u", [V + PAD, D], fp8, kind="ExternalInput")
    emb_w = nc.dram_tensor("emb_w", [V + PAD, D], bf16, kind="ExternalInput")
    ixu_d = nc.dram_tensor("ixu", [P, N_CHUNK * C], mybir.dt.int32, kind="ExternalInput")
    ixw_d = nc.dram_tensor("ixw", [P, N_CHUNK * J], mybir.dt.int32, kind="ExternalInput")
    ident_d = nc.dram_tensor("ident2", [P, 2 * P], fp8, kind="ExternalInput")
    loss = nc.dram_tensor("loss_part", [1, 1], fp32, kind="ExternalOutput")
    if debug_dump:
        dbg_h = nc.dram_tensor("dbg_h", [P, N_CHUNK * D], bf16, kind="ExternalOutput")
        dbg_sc = nc.dram_tensor("dbg_sc", [P, N_CHUNK * J], fp32, kind="ExternalOutput")
        dbg_ug = nc.dram_tensor("dbg_ug", [P, N_CHUNK * C * D], fp8, kind="ExternalOutput")
        dbg_wg = nc.dram_tensor("dbg_wg", [P, N_CHUNK * J * D], bf16, kind="ExternalOutput")

    with tile.TileContext(nc) as tc:
        with (
            tc.tile_pool(name="idx", bufs=1) as idx_pool,
            tc.tile_pool(name="gb", bufs=1) as gb_pool,
            tc.tile_pool(name="m", bufs=3) as m_pool,
            tc.tile_pool(name="fin", bufs=1) as fin_pool,
            tc.tile_pool(name="ps", bufs=3, space="PSUM") as ps_pool,
            tc.tile_pool(name="ps1", bufs=1, space="PSUM") as ps1_pool,
        ):
            ones = fin_pool.tile([P, 1], fp32, tag="ones")
            nc.gpsimd.memset(ones[:], 1.0)

            # --- warm the Exp act table during the preamble ---
            warm_sp = fin_pool.tile([P, 1], fp32, tag="warm_sp")
            nc.scalar.activation(out=warm_sp[:], in_=ones[:], func=EXPF)

            # --- index + stacked-identity loads (SP hw-dge queue) ---
            uix = idx_pool.tile([P, N_CHUNK * C], mybir.dt.int32, tag="uix")
            nc.sync.dma_start(out=uix[:], in_=ixu_d[:])
            wix = idx_pool.tile([P, N_CHUNK * J], mybir.dt.int32, tag="wix")
            nc.scalar.dma_start(out=wix[:], in_=ixw_d[:])
            ident2 = fin_pool.tile([P, 2 * P], fp8, tag="ident2")
            nc.scalar.dma_start(out=ident2[:], in_=ident_d[:])
            id3 = ident2[:].rearrange("p (t i) -> p t i", t=2)

            # --- PE warmup: keep the array out of its low p-state ---
            warm_ps = ps1_pool.tile([P, P], fp32, space="PSUM")
            for _ in range(12):
                nc.tensor.matmul(
                    out=warm_ps[:], lhsT=id3, rhs=id3,
                    start=True, stop=True,
                    perf_mode=mybir.MatmulPerfMode.DoubleRow,
                )

            # --- gathered data (whole-kernel tiles; slices per group) ---
            ug_all = gb_pool.tile([P, N_CHUNK * C * D], fp8, tag="ug")
            wg_all = gb_pool.tile([P, N_CHUNK * J * D], bf16, tag="wg")
            h_all = gb_pool.tile([P, N_CHUNK * D], bf16, tag="h")
            sc_all = fin_pool.tile([P, N_CHUNK * J], fp32, tag="sc")
            # exp(-pos) in cols [0:16), exp(+neg) in cols [16:96)
            ex_all = fin_pool.tile([P, N_CHUNK * J], fp32, tag="ex_all")

            # --- gather issue (Pool): gen serialization paces everything ---
            NU, NH, NW = len(U_GROUPS), len(H_GROUPS), len(W_GROUPS)
            ustarts = [sum(U_GROUPS[:g]) for g in range(NU)]
            hstarts = [sum(H_GROUPS[:g]) for g in range(NH)]
            wstarts = [sum(W_GROUPS[:g]) for g in range(NW)]

            def gen_u(g):
                n, c0 = U_GROUPS[g], ustarts[g]
                nc.gpsimd.indirect_dma_start(
                    out=ug_all[:, c0 * C * D : (c0 + n) * C * D],
                    out_offset=None,
                    in_=emb_u[:],
                    in_offset=bass.IndirectOffsetOnAxis(
                        ap=uix[:, c0 * C : (c0 + n) * C], axis=0
                    ),
                )

            def gen_w(g):
                n, c0 = W_GROUPS[g], wstarts[g]
                nc.gpsimd.indirect_dma_start(
                    out=wg_all[:, c0 * J * D : (c0 + n) * J * D],
                    out_offset=None,
                    in_=emb_w[:],
                    in_offset=bass.IndirectOffsetOnAxis(
                        ap=wix[:, c0 * J : (c0 + n) * J], axis=0
                    ),
                )

            for which, g in [("u", 0), ("w", 0), ("u", 1), ("w", 1), ("u", 2),
                             ("w", 2), ("u", 3), ("w", 3), ("u", 4), ("w", 4),
                             ("w", 5)]:
                (gen_u if which == "u" else gen_w)(g)

            # --- PE h accumulation (fp8 DoubleRow: 2 ctx rows/pass) ---
            # ug is j-major per U group: offset = cu0*C*D + j*nu*D + c*D + d,
            # so a ctx-pair slice is [p][2][n*D] with contiguous cols.
            def u_group_of(c0, n):
                for gu in range(NU):
                    if ustarts[gu] <= c0 and c0 + n <= ustarts[gu] + U_GROUPS[gu]:
                        return gu
                raise AssertionError

            for g in range(NH):
                n, c0 = H_GROUPS[g], hstarts[g]
                gu = u_group_of(c0, n)
                nu, cu0 = U_GROUPS[gu], ustarts[gu]
                base = cu0 * C * D
                hps = ps_pool.tile([P, 4 * D], fp32, space="PSUM", tag="hps")
                for t in range(C // 2):
                    pair = ug_all[
                        :, base + 2 * t * nu * D : base + (2 * t + 2) * nu * D
                    ].rearrange("p (j e) -> p j e", j=2)
                    nc.tensor.matmul(
                        out=hps[:, 0 : n * D],
                        lhsT=id3,
                        rhs=pair[:, :, (c0 - cu0) * D : (c0 - cu0 + n) * D],
                        start=(t == 0),
                        stop=(t == C // 2 - 1),
                        perf_mode=mybir.MatmulPerfMode.DoubleRow,
                    )
                nc.scalar.activation(
                    out=h_all[:, c0 * D : (c0 + n) * D],
                    in_=hps[:, 0 : n * D],
                    func=CP,
                    scale=1.0 / U_SCALE,
                )

            # --- DVE mult/folds/reduce + ACT exps per W group ---
            for g in range(NW):
                n, c0 = W_GROUPS[g], wstarts[g]
                w4 = wg_all[:, c0 * J * D : (c0 + n) * J * D].rearrange(
                    "p (c j d) -> p c j d", c=n, j=J
                )
                h4 = h_all[:, c0 * D : (c0 + n) * D].rearrange(
                    "p (c d) -> p c d", c=n
                )
                m = m_pool.tile([P, 4 * J * D], bf16, tag="m")
                m4 = m[:, 0 : n * J * D].rearrange("p (c j d) -> p c j d", c=n, j=J)
                nc.vector.tensor_mul(
                    out=m4,
                    in0=w4,
                    in1=h4[:, :, None, :].broadcast_to([P, n, J, D]),
                )
                for w_ in (64, 32, 16):
                    nc.vector.tensor_add(
                        out=m4[:, :, :, 0:w_],
                        in0=m4[:, :, :, 0:w_],
                        in1=m4[:, :, :, w_ : 2 * w_],
                    )
                sc3 = sc_all[:, c0 * J : (c0 + n) * J].rearrange(
                    "p (c j) -> p c j", j=J
                )
                nc.vector.tensor_reduce(
                    out=sc3, in_=m4[:, :, :, 0:16], axis=X, op=ADD
                )
                nc.scalar.activation(
                    out=ex_all[:, c0 : c0 + n],
                    in_=sc3[:, :, 0:1],
                    func=EXPF,
                    scale=-1.0,
                )
                nc.scalar.activation(
                    out=ex_all[:, N_CHUNK + c0 * K : N_CHUNK + (c0 + n) * K],
                    in_=sc3[:, :, 1:J],
                    func=EXPF,
                    scale=1.0,
                )

            # --- finale: ln(1+ex) summed per partition, then collapse.
            # (A [P,1] output DMA would need 128 4-byte descriptors whose
            # completion lags ~7us; the ones-matmul keeps it one descriptor.)
            sp = fin_pool.tile([P, N_CHUNK * J], fp32, tag="sp")
            lp1 = fin_pool.tile([P, 1], fp32, tag="lp1")
            nc.scalar.activation(
                out=sp[:], in_=ex_all[:], func=LNF, bias=1.0, accum_out=lp1[:]
            )
            acc = ps1_pool.tile([1, 1], fp32, space="PSUM")
            nc.tensor.matmul(out=acc[:], lhsT=ones[:], rhs=lp1[:], start=True, stop=True)
            out_sb = fin_pool.tile([1, 1], fp32, tag="out")
            nc.vector.tensor_copy(out=out_sb[:], in_=acc[:])
            nc.sync.dma_start(out=loss[:], in_=out_sb[:])
            if debug_dump:
                nc.sync.dma_start(out=dbg_h[:], in_=h_all[:])
                nc.sync.dma_start(out=dbg_sc[:], in_=sc_all[:])
                nc.sync.dma_start(out=dbg_ug[:], in_=ug_all[:])
                nc.sync.dma_start(out=dbg_wg[:], in_=wg_all[:])

    nc.compile()
    return nc


def _get_nc():
    if "nc" not in _NC_CACHE:
        _NC_CACHE["nc"] = _build_bass()
    return _NC_CACHE["nc"]


def _make_in_maps(pos_u, pos_w, neg_w, u_emb, w_emb):
    pos_u = np.asarray(pos_u).astype(np.int32)
    pos_w = np.asarray(pos_w).astype(np.int32)
    neg_w = np.asarray(neg_w).astype(np.int32)
    u_emb = np.asarray(u_emb, dtype=np.float32)
    w_emb = np.asarray(w_emb, dtype=np.float32)

    emb_u = np.zeros((V + PAD, D), dtype=ml_dtypes.float8_e4m3)
    emb_u[:V] = (u_emb * U_SCALE).astype(ml_dtypes.float8_e4m3)
    emb_w = np.zeros((V + PAD, D), dtype=ml_dtypes.bfloat16)
    emb_w[:V] = w_emb.astype(ml_dtypes.bfloat16)
    ident2 = np.concatenate([np.eye(P), np.eye(P)], axis=1).astype(
        ml_dtypes.float8_e4m3
    )

    in_maps = []
    for i in range(N_CORES):
        sl = slice(i * B_LOC, (i + 1) * B_LOC)
        # batch row b -> (chunk c = b // 128, partition p = b % 128).
        # u indices are j-major within each U gather group.
        ucj = pos_u[sl].reshape(N_CHUNK, P, C).transpose(1, 2, 0)  # [P, C, N_CHUNK]
        parts = []
        cu0 = 0
        for nu in U_GROUPS:
            parts.append(ucj[:, :, cu0 : cu0 + nu].reshape(P, C * nu))
            cu0 += nu
        uix = np.concatenate(parts, axis=1)
        wrows = np.concatenate([pos_w[sl, None], neg_w[sl]], axis=1)  # [B_LOC, 6]
        wix = (
            wrows.reshape(N_CHUNK, P, J).transpose(1, 0, 2).reshape(P, N_CHUNK * J)
        )
        in_maps.append(
            {
                "emb_u": emb_u,
                "emb_w": emb_w,
                "ixu": np.ascontiguousarray(uix),
                "ixw": np.ascontiguousarray(wix),
                "ident2": ident2,
            }
        )
    return in_maps


def _install_axon_profile_shim():
    """Provide antenv.axon_hooks (missing in this image) so trace=True can
    capture NTFF profiles via the axon PJRT .so, and keep trace artifacts
    local instead of uploading to a bucket."""
    import contextlib
    import ctypes
    import types

    import concourse.bass_utils as bu

    bu.upload_artifacts = lambda tmpdir: tmpdir

    try:
        from antenv.axon_hooks import get_axon_ntff_profile_hook  # noqa: F401

        return
    except ImportError:
        pass

    mod = types.ModuleType("antenv.axon_hooks")
    holder = {}
    mod.set_axon_ntff_profile_hook = lambda h: holder.__setitem__("h", h)
    mod.get_axon_ntff_profile_hook = lambda: holder.get("h")
    sys.modules["antenv.axon_hooks"] = mod
    import antenv

    antenv.axon_hooks = mod

    so_path = "/opt/axon/libaxon_pjrt.so"
    lib = ctypes.CDLL(so_path)
    if not hasattr(lib, "axon_start_nrt_profile"):
        return
    lib.axon_start_nrt_profile.argtypes = [
        ctypes.POINTER(ctypes.c_int64),
        ctypes.c_size_t,
    ]
    lib.axon_start_nrt_profile.restype = ctypes.c_int64
    lib.axon_stop_nrt_profile.argtypes = [ctypes.c_char_p]
    lib.axon_stop_nrt_profile.restype = ctypes.c_int64

    @contextlib.contextmanager
    def _hook(output_dir, device_ids):
        import jax

        jax.devices()
        if device_ids:
            ids = (ctypes.c_int64 * len(device_ids))(*device_ids)
            rc = lib.axon_start_nrt_profile(ids, len(device_ids))
        else:
            rc = lib.axon_start_nrt_profile(None, 0)
        if rc != 0:
            raise RuntimeError(f"axon_start_nrt_profile rc={rc}")
        try:
            yield
        finally:
            n = lib.axon_stop_nrt_profile(str(output_dir).encode())
            print(f"profile: {n} file(s) written to {output_dir}")

    mod.set_axon_ntff_profile_hook(_hook)


def _run(in_maps, trace=False):
    if trace:
        _install_axon_profile_shim()
    nc = _get_nc()
    return run_bass_kernel_spmd(nc, in_maps, list(range(N_CORES)), trace=trace)


def kernel(pos_u, pos_w, neg_w, u_emb, w_emb):
    in_maps = _make_in_maps(pos_u, pos_w, neg_w, u_emb, w_emb)
    bkr = _run(in_maps, trace=False)
    total = 0.0
    for r in bkr.results:
        total += float(r["loss_part"].astype(np.float64).sum())
    return np.float32(total)


def kernel_traced(pos_u, pos_w, neg_w, u_emb, w_emb):
    """Like kernel() but returns (loss, BassKernelResults) with HW profile."""
    in_maps = _make_in_maps(pos_u, pos_w, neg_w, u_emb, w_emb)
    bkr = _run(in_maps, trace=True)
    total = 0.0
    for r in bkr.results:
        total += float(r["loss_part"].astype(np.float64).sum())
    return np.float32(total), bkr
